# revision 1
# baseline (speedup 1.0000x reference)
"""Trainium2 Bass kernel for nn_CriterionPairWiseforWholeFeatAfterPool.

Computation (reference): select feat_ind slice -> MaxPool2d with kernel
(H/2, W/2) producing a 2x2 pooled map per (sample, channel) -> L2-normalize
over channels -> per-sample 4x4 gram over the pooled spatial positions ->
scalar MSE-style loss between teacher/student grams.

Strategy (data-parallel, per sharding hint): shard the batch axis B=16 across
8 NeuronCores (2 samples/core).  On each core, stream both feature tensors
(2 samples x 256 ch x 128 x 128 f32 = 64 MiB) through SBUF and reduce each
64x64 max-pool window with the vector engine (free-axis reduce_max, channels
on partitions).  Each core emits its pooled features (128 partitions x 32
values = 16 KiB).  The tiny remaining math (per-sample 4x4 gram of a 256x4
matrix, normalization, final sum - a few thousand flops) runs on host, which
also plays the role of the final all-reduce of the per-core partial results.

The kernel is memory-bound: 512 MiB total HBM reads across the chip,
~358 GB/s per-core HBM bandwidth -> ~187 us roofline.
"""

import numpy as np

import concourse.bacc as bacc
import concourse.mybir as mybir
from concourse import tile
from concourse.bass_utils import run_bass_kernel_spmd

N_CORES = 8
P = 128           # SBUF partitions
B_LOC = 2         # samples per core (16 / 8)
C = 256           # channels
H = 128
W = 128
BAND = 64         # pooling-window rows per streamed tile
FREE = BAND * W   # f32 elements per partition per tile (32 KiB)
N_COLS = B_LOC * 2 * (C // P) * (H // BAND) * 2  # = 32 pooled values/partition

_NC = None


def _build_nc():
    """Build + compile the per-core SPMD Bass program (same NEFF on all cores)."""
    nc = bacc.Bacc("TRN2", target_bir_lowering=False, debug=False,
                   num_devices=N_CORES)
    s = nc.dram_tensor("s", [B_LOC, C, H, W], mybir.dt.float32,
                       kind="ExternalInput").ap()
    t = nc.dram_tensor("t", [B_LOC, C, H, W], mybir.dt.float32,
                       kind="ExternalInput").ap()
    out = nc.dram_tensor("pooled", [P, N_COLS], mybir.dt.float32,
                         kind="ExternalOutput").ap()

    with tile.TileContext(nc) as tc:
        with tc.tile_pool(name="data", bufs=4) as data_pool, \
             tc.tile_pool(name="acc", bufs=1) as acc_pool:
            pooled = acc_pool.tile([P, N_COLS], mybir.dt.float32)
            col = 0
            for b in range(B_LOC):
                for x in (s, t):
                    for cb in range(C // P):
                        for band in range(H // BAND):
                            dtile = data_pool.tile([P, FREE], mybir.dt.float32)
                            src = x[b, cb * P:(cb + 1) * P,
                                    band * BAND:(band + 1) * BAND, :]
                            nc.sync.dma_start(
                                out=dtile[:, :],
                                in_=src.rearrange("c h w -> c (h w)"))
                            # free dim is (h, w) row-major; expose the two
                            # 64-wide halves as an outer axis and reduce the
                            # 64x64 window per half.
                            view = dtile[:, :].rearrange(
                                "c (h j w) -> c j h w", h=BAND, j=2, w=64)
                            nc.vector.reduce_max(
                                pooled[:, col:col + 2], view,
                                axis=mybir.AxisListType.XY)
                            col += 2
            nc.sync.dma_start(out=out, in_=pooled[:, :])
    nc.compile()
    return nc


def get_nc():
    global _NC
    if _NC is None:
        _NC = _build_nc()
    return _NC


def make_in_maps(fS, fT):
    """Per-core input dicts: batch-sharded contiguous slices."""
    return [{"s": np.ascontiguousarray(fS[B_LOC * i:B_LOC * (i + 1)]),
             "t": np.ascontiguousarray(fT[B_LOC * i:B_LOC * (i + 1)])}
            for i in range(N_CORES)]


def finish(pooled_list):
    """Host epilogue: reassemble pooled features, gram + normalize + loss."""
    B = B_LOC * N_CORES
    fS = np.empty((B, C, 4), np.float64)
    fT = np.empty((B, C, 4), np.float64)
    for i, arr in enumerate(pooled_list):
        # columns are (b_local, tensor, ch_block, band_i, half_j)
        a = np.asarray(arr).reshape(P, B_LOC, 2, C // P, 2, 2)
        for bl in range(B_LOC):
            b = i * B_LOC + bl
            for cb in range(C // P):
                ch = slice(cb * P, (cb + 1) * P)
                fS[b, ch, :] = a[:, bl, 0, cb].reshape(P, 4)
                fT[b, ch, :] = a[:, bl, 1, cb].reshape(P, 4)

    def sim(f):
        G = np.einsum('bcm,bcn->bmn', f, f)
        d = np.sqrt(np.einsum('bmm->bm', G)) + 1e-8
        return G / (d[:, :, None] * d[:, None, :])

    loss = ((sim(fT) - sim(fS)) ** 2).sum() / (4 * 4) / B
    return np.float32(loss)


def run_device(fS, fT, **spmd_kwargs):
    """Run the compiled program on the 8 cores; returns (pooled_list, results)."""
    res = run_bass_kernel_spmd(get_nc(), make_in_maps(fS, fT),
                               core_ids=list(range(N_CORES)), **spmd_kwargs)
    pooled_list = [res.results[i]["pooled"] for i in range(N_CORES)]
    return pooled_list, res


def kernel(preds_S, preds_T, feat_ind):
    fi = int(np.asarray(feat_ind))
    fS = np.ascontiguousarray(np.asarray(preds_S)[fi], dtype=np.float32)
    fT = np.ascontiguousarray(np.asarray(preds_T)[fi], dtype=np.float32)
    pooled_list, _ = run_device(fS, fT)
    return finish(pooled_list)


# revision 4
# speedup vs baseline: 1.0454x; 1.0454x over previous
"""Trainium2 Bass kernel for nn_CriterionPairWiseforWholeFeatAfterPool.

Computation (reference): select feat_ind slice -> MaxPool2d with kernel
(H/2, W/2) producing a 2x2 pooled map per (sample, channel) -> L2-normalize
over channels -> per-sample 4x4 gram over the pooled spatial positions ->
scalar MSE-style loss between teacher/student grams.

Strategy (data-parallel, per the sharding hint): shard the batch axis B=16
across 8 NeuronCores (2 samples/core).  Each core streams its two feature
shards (2 samples x 256 ch x 128 x 128 f32 = 64 MiB) HBM->SBUF with
channels on partitions and reduces every 64x64 max-pool window on the
vector engine (free-axis reduce_max over a strided quadrant view).  Each
core emits its pooled features (128 partitions x 38 cols, 19 KiB).  The
tiny epilogue (per-sample 4x4 gram of a 256x4 matrix, normalization from
the gram diagonal, final sum == the all-reduce of per-core partials) runs
on host in a few microseconds of numpy.

The kernel is memory-bound and measured at the bare-DMA-stream floor of
this hardware: a DMA-only NEFF moving the same 64 MiB/core takes ~176 us;
the full kernel runs ~177 us (HBM roofline 512 MiB / chip).

Implementation: raw Bass blocks (no Tile framework) with hand-rolled
double buffering - NBUF slots, one DMA-completion semaphore per slot (at
most one in-flight DMA per semaphore), and a reduce-counter semaphore for
write-after-read slot protection.  The final pooling band is streamed as
4 slim chunks so the last reduce (which gates the kernel tail) is short;
the host folds the partial maxes.
"""

import contextlib

import numpy as np

import concourse.bacc as bacc
import concourse.mybir as mybir
from concourse.bass_utils import run_bass_kernel_spmd

N_CORES = 8
P = 128           # SBUF partitions
B_LOC = 2         # samples per core (16 / 8)
C = 256           # channels
H = 128
W = 128
BAND = 64         # pooling-window rows per streamed tile (4 MiB tiles)
FREE = BAND * W   # f32 elements per partition per tile (32 KiB)
TAIL_SPLIT = 4    # last band streamed as 4 slim chunks (short final reduce)
N_TILES = B_LOC * 2 * (C // P) * (H // BAND)            # 16 full-band tiles
N_COLS = (N_TILES - 1) * 2 + TAIL_SPLIT * 2             # 38 pooled cols
NBUF = 5          # SBUF slots (5 x 32 KiB/partition = 160 KiB of 192)

_NC = None


def _build_nc():
    """Build + compile the per-core SPMD Bass program (same NEFF on all cores)."""
    nc = bacc.Bacc("TRN2", target_bir_lowering=False, debug=False,
                   num_devices=N_CORES)
    s = nc.dram_tensor("s", [B_LOC, C, H, W], mybir.dt.float32,
                       kind="ExternalInput").ap()
    t = nc.dram_tensor("t", [B_LOC, C, H, W], mybir.dt.float32,
                       kind="ExternalInput").ap()
    out = nc.dram_tensor("pooled", [P, N_COLS], mybir.dt.float32,
                         kind="ExternalOutput").ap()

    order = [(x, b, cb, band)
             for b in range(B_LOC) for x in (s, t)
             for cb in range(C // P) for band in range(H // BAND)]
    rows_tail = BAND // TAIL_SPLIT

    # transfer list: (2-D dram source AP, free elems, rows covered)
    xfers = []
    for x, b, cb, band in order[:-1]:
        src = x[b, cb * P:(cb + 1) * P, band * BAND:(band + 1) * BAND, :]
        xfers.append((src.rearrange("c h w -> c (h w)"), FREE, BAND))
    x, b, cb, band = order[-1]
    for k in range(TAIL_SPLIT):
        r0 = band * BAND + k * rows_tail
        src = x[b, cb * P:(cb + 1) * P, r0:r0 + rows_tail, :]
        xfers.append((src.rearrange("c h w -> c (h w)"), rows_tail * W,
                      rows_tail))
    n = len(xfers)

    with contextlib.ExitStack() as ctx:
        bufs = [ctx.enter_context(
            nc.sbuf_tensor(f"buf{i}", [P, FREE], mybir.dt.float32))
            for i in range(NBUF)]
        pooled = ctx.enter_context(
            nc.sbuf_tensor("pooled_sb", [P, N_COLS], mybir.dt.float32))
        # one DMA-completion semaphore per buffer slot: at most one in-flight
        # DMA per semaphore (slot reuse is serialized through red_sem), so
        # concurrent DMAs never race on the same semaphore
        dma_sems = [ctx.enter_context(nc.semaphore(f"dma_sem{i}"))
                    for i in range(NBUF)]
        out_sem = ctx.enter_context(nc.semaphore("out_sem"))
        red_sem = ctx.enter_context(nc.semaphore("red_sem"))
        block = ctx.enter_context(nc.Block())

        @block.sync
        def _(sync):
            for i, (src, free, _h) in enumerate(xfers):
                if i >= NBUF:
                    # slot reuse: wait until the reduce of tile i-NBUF is done
                    sync.wait_ge(red_sem, i - NBUF + 1)
                sync.dma_start(
                    bufs[i % NBUF][:, :free], src).then_inc(
                        dma_sems[i % NBUF], 16)
            sync.wait_ge(red_sem, n)
            sync.dma_start(out, pooled[:, :]).then_inc(out_sem, 16)
            sync.wait_ge(out_sem, 16)

        @block.vector
        def _(vector):
            for i, (_src, free, h) in enumerate(xfers):
                vector.wait_ge(dma_sems[i % NBUF], 16 * (i // NBUF + 1))
                # free dim is (h, w) row-major; expose the two 64-wide halves
                # as an outer axis, reduce the h x 64 window per half
                view = bufs[i % NBUF][:, :free].rearrange(
                    "c (h j w) -> c j h w", h=h, j=2, w=64)
                vector.tensor_reduce(
                    pooled[:, 2 * i:2 * i + 2], view,
                    axis=mybir.AxisListType.XY,
                    op=mybir.AluOpType.max).then_inc(red_sem, 1)

    nc.compile()
    return nc


def get_nc():
    global _NC
    if _NC is None:
        _NC = _build_nc()
    return _NC


def make_in_maps(fS, fT):
    """Per-core input dicts: batch-sharded contiguous slices."""
    return [{"s": np.ascontiguousarray(fS[B_LOC * i:B_LOC * (i + 1)]),
             "t": np.ascontiguousarray(fT[B_LOC * i:B_LOC * (i + 1)])}
            for i in range(N_CORES)]


def finish(pooled_list):
    """Host epilogue: reassemble pooled features, gram + normalize + loss."""
    B = B_LOC * N_CORES
    fS = np.empty((B, C, 4), np.float64)
    fT = np.empty((B, C, 4), np.float64)
    order = [(xi, bl, cb, band)
             for bl in range(B_LOC) for xi in range(2)
             for cb in range(C // P) for band in range(H // BAND)]
    for i, arr in enumerate(pooled_list):
        a = np.asarray(arr)  # [P, N_COLS]
        f = (fS, fT)
        for k, (xi, bl, cb, band) in enumerate(order[:-1]):
            # cols 2k,2k+1 = quadrants (band, 0..1) of this channel block
            f[xi][i * B_LOC + bl, cb * P:(cb + 1) * P,
                  band * 2:band * 2 + 2] = a[:, 2 * k:2 * k + 2]
        xi, bl, cb, band = order[-1]  # tail-split band: fold partial maxes
        v = a[:, 2 * (len(order) - 1):].reshape(P, TAIL_SPLIT, 2).max(axis=1)
        f[xi][i * B_LOC + bl, cb * P:(cb + 1) * P,
              band * 2:band * 2 + 2] = v

    def sim(f):
        G = np.einsum('bcm,bcn->bmn', f, f)
        d = np.sqrt(np.einsum('bmm->bm', G)) + 1e-8
        return G / (d[:, :, None] * d[:, None, :])

    loss = ((sim(fT) - sim(fS)) ** 2).sum() / (4 * 4) / B
    return np.float32(loss)


def run_device(fS, fT, **spmd_kwargs):
    """Run the compiled program on the 8 cores; returns (pooled_list, results)."""
    res = run_bass_kernel_spmd(get_nc(), make_in_maps(fS, fT),
                               core_ids=list(range(N_CORES)), **spmd_kwargs)
    pooled_list = [res.results[i]["pooled"] for i in range(N_CORES)]
    return pooled_list, res


def kernel(preds_S, preds_T, feat_ind):
    fi = int(np.asarray(feat_ind))
    fS = np.ascontiguousarray(np.asarray(preds_S)[fi], dtype=np.float32)
    fT = np.ascontiguousarray(np.asarray(preds_T)[fi], dtype=np.float32)
    pooled_list, _ = run_device(fS, fT)
    return finish(pooled_list)


# revision 5
# speedup vs baseline: 1.0481x; 1.0026x over previous
"""Trainium2 Bass kernel for nn_CriterionPairWiseforWholeFeatAfterPool.

Computation (reference): select feat_ind slice -> MaxPool2d with kernel
(H/2, W/2) producing a 2x2 pooled map per (sample, channel) -> L2-normalize
over channels -> per-sample 4x4 gram over the pooled spatial positions ->
scalar MSE-style loss between teacher/student grams.

Strategy (data-parallel, per the sharding hint): shard the batch axis B=16
across 8 NeuronCores (2 samples/core).  Each core streams its two feature
shards (2 samples x 256 ch x 128 x 128 f32 = 64 MiB) HBM->SBUF with
channels on partitions and reduces every 64x64 max-pool window on the
vector engine (free-axis reduce_max over a strided quadrant view).  Each
core emits its pooled features (128 partitions x 38 cols, 19 KiB).  The
tiny epilogue (per-sample 4x4 gram of a 256x4 matrix, normalization from
the gram diagonal, final sum == the all-reduce of per-core partials) runs
on host in a few microseconds of numpy.

The kernel is memory-bound and measured at the bare-DMA-stream floor of
this hardware: a DMA-only NEFF moving the same 64 MiB/core takes ~176 us;
the full kernel runs ~177 us (HBM roofline 512 MiB / chip).

Implementation: raw Bass blocks (no Tile framework) with hand-rolled
double buffering - NBUF slots, one DMA-completion semaphore per slot (at
most one in-flight DMA per semaphore), and a reduce-counter semaphore for
write-after-read slot protection.  The final pooling band is streamed as
4 slim chunks so the last reduce (which gates the kernel tail) is short;
the host folds the partial maxes.
"""

import contextlib

import numpy as np

import concourse.bacc as bacc
import concourse.mybir as mybir
from concourse.bass_utils import run_bass_kernel_spmd

N_CORES = 8
P = 128           # SBUF partitions
B_LOC = 2         # samples per core (16 / 8)
C = 256           # channels
H = 128
W = 128
BAND = 64         # pooling-window rows per streamed tile (4 MiB tiles)
FREE = BAND * W   # f32 elements per partition per tile (32 KiB)
TAIL_SPLIT = 4    # last band streamed as 4 slim chunks (short final reduce)
N_TILES = B_LOC * 2 * (C // P) * (H // BAND)            # 16 full-band tiles
N_COLS = (N_TILES - 1) * 2 + TAIL_SPLIT * 2             # 38 pooled cols
NBUF = 5          # SBUF slots (5 x 32 KiB/partition = 160 KiB of 192)

_NC = None


def _build_nc():
    """Build + compile the per-core SPMD Bass program (same NEFF on all cores)."""
    nc = bacc.Bacc("TRN2", target_bir_lowering=False, debug=False,
                   num_devices=N_CORES)
    s = nc.dram_tensor("s", [B_LOC, C, H, W], mybir.dt.float32,
                       kind="ExternalInput").ap()
    t = nc.dram_tensor("t", [B_LOC, C, H, W], mybir.dt.float32,
                       kind="ExternalInput").ap()
    out = nc.dram_tensor("pooled", [P, N_COLS], mybir.dt.float32,
                         kind="ExternalOutput").ap()

    order = [(x, b, cb, band)
             for b in range(B_LOC) for x in (s, t)
             for cb in range(C // P) for band in range(H // BAND)]
    rows_tail = BAND // TAIL_SPLIT

    # transfer list: (2-D dram source AP, free elems, rows covered)
    xfers = []
    for x, b, cb, band in order[:-1]:
        src = x[b, cb * P:(cb + 1) * P, band * BAND:(band + 1) * BAND, :]
        xfers.append((src.rearrange("c h w -> c (h w)"), FREE, BAND))
    x, b, cb, band = order[-1]
    for k in range(TAIL_SPLIT):
        r0 = band * BAND + k * rows_tail
        src = x[b, cb * P:(cb + 1) * P, r0:r0 + rows_tail, :]
        xfers.append((src.rearrange("c h w -> c (h w)"), rows_tail * W,
                      rows_tail))
    n = len(xfers)

    with contextlib.ExitStack() as ctx:
        bufs = [ctx.enter_context(
            nc.sbuf_tensor(f"buf{i}", [P, FREE], mybir.dt.float32))
            for i in range(NBUF)]
        pooled = ctx.enter_context(
            nc.sbuf_tensor("pooled_sb", [P, N_COLS], mybir.dt.float32))
        # one DMA-completion semaphore per buffer slot: at most one in-flight
        # DMA per semaphore (slot reuse is serialized through red_sem), so
        # concurrent DMAs never race on the same semaphore
        dma_sems = [ctx.enter_context(nc.semaphore(f"dma_sem{i}"))
                    for i in range(NBUF)]
        out_sem = ctx.enter_context(nc.semaphore("out_sem"))
        red_sem = ctx.enter_context(nc.semaphore("red_sem"))
        block = ctx.enter_context(nc.Block())

        @block.sync
        def _(sync):
            for i, (src, free, _h) in enumerate(xfers):
                if i >= NBUF:
                    # slot reuse: wait until the reduce of tile i-NBUF is done
                    sync.wait_ge(red_sem, i - NBUF + 1)
                sync.dma_start(
                    bufs[i % NBUF][:, :free], src).then_inc(
                        dma_sems[i % NBUF], 16)
            sync.wait_ge(red_sem, n)
            sync.dma_start(out, pooled[:, :]).then_inc(out_sem, 16)
            sync.wait_ge(out_sem, 16)

        @block.vector
        def _(vector):
            for i, (_src, free, h) in enumerate(xfers):
                vector.wait_ge(dma_sems[i % NBUF], 16 * (i // NBUF + 1))
                # free dim is (h, w) row-major; expose the two 64-wide halves
                # as an outer axis, reduce the h x 64 window per half
                view = bufs[i % NBUF][:, :free].rearrange(
                    "c (h j w) -> c j h w", h=h, j=2, w=64)
                vector.tensor_reduce(
                    pooled[:, 2 * i:2 * i + 2], view,
                    axis=mybir.AxisListType.XY,
                    op=mybir.AluOpType.max).then_inc(red_sem, 1)

    nc.compile()
    return nc


def get_nc():
    global _NC
    if _NC is None:
        _NC = _build_nc()
    return _NC


def make_in_maps(fS, fT):
    """Per-core input dicts: batch-sharded contiguous slices."""
    return [{"s": np.ascontiguousarray(fS[B_LOC * i:B_LOC * (i + 1)]),
             "t": np.ascontiguousarray(fT[B_LOC * i:B_LOC * (i + 1)])}
            for i in range(N_CORES)]


def finish(pooled_list):
    """Host epilogue: reassemble pooled features, gram + normalize + loss."""
    B = B_LOC * N_CORES
    fS = np.empty((B, C, 4), np.float64)
    fT = np.empty((B, C, 4), np.float64)
    order = [(xi, bl, cb, band)
             for bl in range(B_LOC) for xi in range(2)
             for cb in range(C // P) for band in range(H // BAND)]
    for i, arr in enumerate(pooled_list):
        a = np.asarray(arr)  # [P, N_COLS]
        f = (fS, fT)
        for k, (xi, bl, cb, band) in enumerate(order[:-1]):
            # cols 2k,2k+1 = quadrants (band, 0..1) of this channel block
            f[xi][i * B_LOC + bl, cb * P:(cb + 1) * P,
                  band * 2:band * 2 + 2] = a[:, 2 * k:2 * k + 2]
        xi, bl, cb, band = order[-1]  # tail-split band: fold partial maxes
        v = a[:, 2 * (len(order) - 1):].reshape(P, TAIL_SPLIT, 2).max(axis=1)
        f[xi][i * B_LOC + bl, cb * P:(cb + 1) * P,
              band * 2:band * 2 + 2] = v

    def sim(f):
        G = np.einsum('bcm,bcn->bmn', f, f)
        d = np.sqrt(np.einsum('bmm->bm', G)) + 1e-8
        return G / (d[:, :, None] * d[:, None, :])

    loss = ((sim(fT) - sim(fS)) ** 2).sum() / (4 * 4) / B
    return np.float32(loss)


def run_device(fS, fT, **spmd_kwargs):
    """Run the compiled program on the 8 cores; returns (pooled_list, results)."""
    res = run_bass_kernel_spmd(get_nc(), make_in_maps(fS, fT),
                               core_ids=list(range(N_CORES)), **spmd_kwargs)
    pooled_list = [res.results[i]["pooled"] for i in range(N_CORES)]
    return pooled_list, res


def kernel(preds_S, preds_T, feat_ind):
    fi = int(np.asarray(feat_ind))
    fS = np.ascontiguousarray(np.asarray(preds_S)[fi], dtype=np.float32)
    fT = np.ascontiguousarray(np.asarray(preds_T)[fi], dtype=np.float32)
    try:
        pooled_list, _ = run_device(fS, fT)
    except Exception:
        # one retry: a cold device occasionally reports a transient
        # NRT execution error on the very first NEFF launch
        pooled_list, _ = run_device(fS, fT)
    return finish(pooled_list)
